# revision 1
# baseline (speedup 1.0000x reference)
"""DCRNN (2x GCNConv + GRU-over-nodes + Linear) on 8 Trainium2 cores.

Strategy
--------
* GCN layers: the normalized adjacency (A+I with D^-1/2 scaling) is built
  densely on the host in fp16 and sharded row-wise across the 8 cores
  (1250 rows/core + a 64-row left halo for the GRU).  Each GCN layer is a
  dense [rows, 10000] @ [10000, 256] fp16 matmul on the TensorEngine, with
  everything kept transposed ([feat, node] layout) so no on-device
  transposes are needed.  One AllGather shares h1 between cores.
* GRU over the 10000-node sequence is strictly sequential in the
  reference.  We solve it with K fixed-point sweeps: gates (r,z,n) are
  evaluated from the previous sweep's hidden state (one big matmul +
  pointwise), then h_t = z_t*h_{t-1} + (1-z_t)*n_t is applied EXACTLY with
  the DVE affine-scan primitive (tensor_tensor_scan).  The per-step decay
  |dh_t/dh_{t-1}| ~ 0.74 makes this converge geometrically; the 64-row
  halo makes the cores fully independent (boundary error ~ 0.74^64).
* Final Linear runs on the node shard; host concatenates the 8 shards.
"""

import numpy as np

NUM_NODES = 10000
IN_FEAT = 64
HID = 256
OUT = 3
CORES = 8
ROWS = NUM_NODES // CORES          # 1250
HALO = 64
L = ROWS + HALO                    # 1314 local sequence length
SWEEPS = 10
MT = 79                            # ceil(10000/128) K-tiles (79*128 = 10112 > 10000)
KP = 128
NLOC = (ROWS + KP - 1) // KP       # 10 local K-tiles (last has 98 rows)
MT2 = MT + NLOC                    # global tiles + appended local-shard tiles
GRP = (MT2 + 3) // 4               # 23 interleave groups of 4 K-tiles

_CACHE = {}


def _chunks(total, step=512):
    return [(c, min(c + step, total)) for c in range(0, total, step)]


def build_program():
    import concourse.bass as bass
    import concourse.mybir as mybir
    import concourse.tile as tile
    from concourse import bacc

    f16 = mybir.dt.float16
    f32 = mybir.dt.float32
    AF = mybir.ActivationFunctionType
    ALU = mybir.AluOpType

    nc = bacc.Bacc("TRN2", num_devices=CORES)

    # K-padded node count (pad rows zero).  The per-core A strip carries the
    # global rows (with this core's own rows zeroed) followed by a copy of
    # this core's rows at a fixed offset, so the K-loop can start on the
    # local rows before the XW2 AllGather lands.
    NPAD = MT * KP
    NPAD2 = MT2 * KP

    # ---- inputs ----
    # A strip, 4-way K-tile interleaved: row (g*128+p), col (j*L+c) holds
    # A_T[node g*512+j*128+p, c] -- 10.5KB contiguous DMA descriptors.
    a2t_d = nc.dram_tensor("a2t", [GRP * KP, 4 * L], f16, kind="ExternalInput")
    xt_d = nc.dram_tensor("xt", [IN_FEAT, NPAD2], f16, kind="ExternalInput")
    w1_d = nc.dram_tensor("w1", [IN_FEAT, HID], f16, kind="ExternalInput")
    w2_d = nc.dram_tensor("w2", [HID, HID], f16, kind="ExternalInput")
    wiht_d = nc.dram_tensor("wiht", [HID, 3 * HID], f16, kind="ExternalInput")
    whht_d = nc.dram_tensor("whht", [HID, 3 * HID], f16, kind="ExternalInput")
    fcwt_d = nc.dram_tensor("fcwt", [HID, OUT], f16, kind="ExternalInput")
    ident_d = nc.dram_tensor("ident", [KP, KP], f16, kind="ExternalInput")
    b1c_d = nc.dram_tensor("b1c", [KP, 2], f32, kind="ExternalInput")
    b2c_d = nc.dram_tensor("b2c", [KP, 2], f32, kind="ExternalInput")
    gib_d = nc.dram_tensor("gib", [KP, 6], f32, kind="ExternalInput")
    bhn_d = nc.dram_tensor("bhn", [KP, 2], f32, kind="ExternalInput")
    fcb_d = nc.dram_tensor("fcb", [KP, 1], f32, kind="ExternalInput")
    patch_d = nc.dram_tensor("patch", [KP, 12], f32, kind="ExternalInput")
    out_d = nc.dram_tensor("out_t", [OUT, ROWS], f32, kind="ExternalOutput")

    with tile.TileContext(nc) as tc:
        with (
            tc.tile_pool(name="const", bufs=1) as cpool,
            tc.tile_pool(name="big", bufs=1) as big,
            tc.tile_pool(name="tmp", bufs=3) as tpool,
            tc.tile_pool(name="psxw", bufs=2, space="PSUM") as psxw,
            tc.tile_pool(name="dram", bufs=1, space="DRAM") as dpool,
        ):
            # ---- load constants ----
            xt_cm = tc.tile_pool(name="xtp", bufs=1)
            xtp = xt_cm.__enter__()
            xt_sb = xtp.tile([IN_FEAT, NPAD2], f16)
            w1_sb = cpool.tile([IN_FEAT, HID], f16)
            w2_sb = cpool.tile([KP, 2, HID], f16)
            wiht_sb = cpool.tile([KP, 2, 3 * HID], f16)
            whht_sb = cpool.tile([KP, 2, 3 * HID], f16)
            fcwt_sb = cpool.tile([KP, 2, OUT], f16)
            ident_sb = cpool.tile([KP, KP], f16)
            b1c_sb = cpool.tile([KP, 2], f32)
            b2c_sb = cpool.tile([KP, 2], f32)
            gib_sb = cpool.tile([KP, 6], f32)
            bhn_sb = cpool.tile([KP, 2], f32)
            fcb_sb = cpool.tile([KP, 1], f32)
            patch_sb = cpool.tile([KP, 12], f32)

            nc.sync.dma_start(ident_sb[:], ident_d[:])
            nc.sync.dma_start(w1_sb[:], w1_d[:])
            nc.scalar.dma_start(xt_sb[:], xt_d[:])
            for k in range(2):
                nc.sync.dma_start(w2_sb[:, k, :], w2_d[k * KP:(k + 1) * KP, :])
                nc.sync.dma_start(wiht_sb[:, k, :], wiht_d[k * KP:(k + 1) * KP, :])
                nc.sync.dma_start(whht_sb[:, k, :], whht_d[k * KP:(k + 1) * KP, :])
                nc.sync.dma_start(fcwt_sb[:, k, :], fcwt_d[k * KP:(k + 1) * KP, :])
            nc.sync.dma_start(b1c_sb[:], b1c_d[:])
            nc.sync.dma_start(b2c_sb[:], b2c_d[:])
            nc.sync.dma_start(gib_sb[:], gib_d[:])
            nc.sync.dma_start(bhn_sb[:], bhn_d[:])
            nc.sync.dma_start(fcb_sb[:], fcb_d[:])
            nc.sync.dma_start(patch_sb[:], patch_d[:])

            psG_cm = tc.tile_pool(name="psG", bufs=1, space="PSUM")
            psG = psG_cm.__enter__()

            # tiny AllGather to absorb the first-collective ncfw setup cost
            # (runs on the TOPSP engines, overlapped with GCN layer 1)
            ccw_in = dpool.tile([CORES, 64], f16)
            ccw_out = dpool.tile([CORES * CORES, 64], f16, addr_space="Shared")
            nc.sync.dma_start(ccw_in[0:KP // 16, :], ident_sb[0:8, 0:64])
            nc.gpsimd.collective_compute(
                "AllGather", mybir.AluOpType.bypass,
                replica_groups=[list(range(CORES))],
                ins=[ccw_in.opt()], outs=[ccw_out.opt()])

            # PE warm-up burst so the HAM clock-gate opens before GCN1
            for i in range(40):
                psd = psxw.tile([KP, 512], f32, tag="xwps", name=f"warm_{i}")
                nc.tensor.matmul(psd[:, :KP], ident_sb[:], ident_sb[:],
                                 start=True, stop=True)

            # ---- XW1 = x @ W1 in natural layout ([node(K), 256]) ----
            xw_sb = big.tile([KP, MT2, HID], f16, tag="xw")
            for m in range(MT2):
                ps = psxw.tile([KP, 512], f32, tag="xwps")
                nc.tensor.matmul(ps[:, :HID], xt_sb[:, m * KP:(m + 1) * KP],
                                 w1_sb[:], start=True, stop=True)
                if m % 2 == 0:
                    nc.scalar.activation(xw_sb[:, m, :], ps[:, :HID], AF.Copy)
                else:
                    nc.vector.tensor_copy(xw_sb[:, m, :], ps[:, :HID])

            xt_cm.__exit__(None, None, None)
            ap_cm = tc.tile_pool(name="astream", bufs=5)
            apool = ap_cm.__enter__()

            # ---- GCN layer 1: h1T_loc = relu(bias + XW1.T @ A1) ----
            chg1 = _chunks(ROWS)
            ps1 = [[psG.tile([KP, 512], f32,
                             tag=f"G{mm * 3 + ci}", name=f"ps1_{mm}_{ci}")
                    for ci in range(3)] for mm in range(2)]
            for g in range(GRP):
                at = apool.tile([KP, 4 * L], f16, tag="a")
                eng = nc.sync if g % 2 == 0 else nc.scalar
                eng.dma_start(at[:], a2t_d[g * KP:(g + 1) * KP, :])
                for j in range(4):
                    k = 4 * g + j
                    if k >= MT2:
                        break
                    for mm in range(2):
                        lhsT = xw_sb[:, k, mm * KP:(mm + 1) * KP]
                        for ci, (c0, c1) in enumerate(chg1):
                            nc.tensor.matmul(
                                ps1[mm][ci][:, :c1 - c0], lhsT,
                                at[:, j * L + HALO + c0:j * L + HALO + c1],
                                start=(k == 0), stop=(k == MT2 - 1))
            h1t_sb = big.tile([KP, 2, ROWS], f16)
            for mm in range(2):
                for ci, (c0, c1) in enumerate(chg1):
                    nc.scalar.activation(h1t_sb[:, mm, c0:c1],
                                         ps1[mm][ci][:, :c1 - c0], AF.Relu,
                                         bias=b1c_sb[:, mm:mm + 1])

            # ---- XW2 shard = h1_loc @ W2 (natural layout), then AllGather XW2 ----
            xw2l_sb = cpool.tile([KP, NLOC, HID], f16)
            for j in range(NLOC):
                rw = min(KP, ROWS - j * KP)
                ps = psxw.tile([KP, 512], f32, tag="xwps")
                for k in range(2):
                    nc.tensor.matmul(ps[:rw, :HID],
                                     h1t_sb[:, k, j * KP:j * KP + rw],
                                     w2_sb[:, k, :],
                                     start=(k == 0), stop=(k == 1))
                if j % 2 == 0:
                    nc.scalar.activation(xw2l_sb[:rw, j, :], ps[:rw, :HID], AF.Copy)
                else:
                    nc.vector.tensor_copy(xw2l_sb[:rw, j, :], ps[:rw, :HID])
            # Two half-feature AllGathers: the mm=0 matmuls of GCN2 need only
            # the first gather, so they overlap the second one.
            bounce_a = dpool.tile([ROWS, KP], f16)
            bounce_b = dpool.tile([ROWS, KP], f16)
            gath_a = dpool.tile([NUM_NODES, KP], f16, addr_space="Shared")
            gath_b = dpool.tile([NUM_NODES, KP], f16, addr_space="Shared")
            for j in range(NLOC):
                rw = min(KP, ROWS - j * KP)
                nc.sync.dma_start(bounce_a[j * KP:j * KP + rw, :],
                                  xw2l_sb[:rw, j, 0:KP])
                nc.sync.dma_start(bounce_b[j * KP:j * KP + rw, :],
                                  xw2l_sb[:rw, j, KP:HID])
            nc.gpsimd.collective_compute(
                "AllGather", mybir.AluOpType.bypass,
                replica_groups=[list(range(CORES))],
                ins=[bounce_a.opt()], outs=[gath_a.opt()])
            nc.gpsimd.collective_compute(
                "AllGather", mybir.AluOpType.bypass,
                replica_groups=[list(range(CORES))],
                ins=[bounce_b.opt()], outs=[gath_b.opt()])
            xw2_sb = big.tile([KP, MT, KP], f16, tag="xw")
            xw2b_sb = big.tile([KP, MT, KP], f16, tag="xwb")
            nc.vector.memset(xw2_sb[:, MT - 1, :], 0.0)
            nc.vector.memset(xw2b_sb[:, MT - 1, :], 0.0)
            for k in range(MT):
                rw = min(KP, NUM_NODES - k * KP)
                eng = nc.sync if k % 2 == 0 else nc.scalar
                eng.dma_start(xw2_sb[:rw, k, :],
                              gath_a[k * KP:k * KP + rw, :])
            for k in range(MT):
                rw = min(KP, NUM_NODES - k * KP)
                eng = nc.sync if k % 2 == 0 else nc.scalar
                eng.dma_start(xw2b_sb[:rw, k, :],
                              gath_b[k * KP:k * KP + rw, :])

            # ---- GCN layer 2 over the extended (halo) shard ----
            # K-loop order: this core's own (appended) rows first, using the
            # locally computed XW2 shard -- runs during the AllGather.
            chg2 = _chunks(L)
            ps2 = [[psG.tile([KP, 512], f32,
                             tag=f"G{mm * 3 + ci}", name=f"ps2_{mm}_{ci}")
                    for ci in range(3)] for mm in range(2)]
            def gcn2_mm(k, at, first, last):
                for mm in range(2):
                    if k >= MT:
                        rw = min(KP, ROWS - (k - MT) * KP)
                        lhsT = xw2l_sb[:rw, k - MT, mm * KP:(mm + 1) * KP]
                    else:
                        rw = KP
                        lhsT = (xw2_sb if mm == 0 else xw2b_sb)[:, k, :]
                    j = k % 4
                    for ci, (c0, c1) in enumerate(chg2):
                        nc.tensor.matmul(
                            ps2[mm][ci][:, :c1 - c0], lhsT,
                            at[:rw, j * L + c0:j * L + c1],
                            start=first, stop=last)

            # appended (local-shard) K-tiles first: groups 19..22, no gather dep
            gtiles = {}
            for g in range(MT // 4, GRP):
                at = apool.tile([KP, 4 * L], f16, tag="a")
                eng = nc.sync if g % 2 == 0 else nc.scalar
                eng.dma_start(at[:], a2t_d[g * KP:(g + 1) * KP, :])
                gtiles[g] = at
                for j in range(4):
                    k = 4 * g + j
                    if MT <= k < MT2:
                        gcn2_mm(k, at, k == MT, False)
            # main global K-tiles (0..75)
            for g in range(MT // 4):
                at = apool.tile([KP, 4 * L], f16, tag="a")
                eng = nc.sync if g % 2 == 0 else nc.scalar
                eng.dma_start(at[:], a2t_d[g * KP:(g + 1) * KP, :])
                for j in range(4):
                    k = 4 * g + j
                    gcn2_mm(k, at, False, False)
            # boundary-group global K-tiles (76..78) from the retained tile
            for k in range(4 * (MT // 4), MT):
                gcn2_mm(k, gtiles[MT // 4], False, k == MT - 1)
            h2t_sb = big.tile([KP, 2, L], f16)
            for mm in range(2):
                for ci, (c0, c1) in enumerate(chg2):
                    nc.scalar.activation(h2t_sb[:, mm, c0:c1],
                                         ps2[mm][ci][:, :c1 - c0], AF.Relu,
                                         bias=b2c_sb[:, mm:mm + 1])

            psG_cm.__exit__(None, None, None)
            ap_cm.__exit__(None, None, None)
            psg_cm = tc.tile_pool(name="ps", bufs=1, space="PSUM")
            pspool = psg_cm.__enter__()

            # ---- GI = W_ih @ h2T + (b_ih [+ b_hh for r,z]) ----
            ch512 = _chunks(L)
            gi_sb = big.tile([KP, 6, L], f16)
            for c0, c1 in ch512:
                psg = [pspool.tile([KP, 512], f32, tag=f"g{m}", name=f"psgi_{m}") for m in range(6)]
                for m in range(6):
                    for k in range(2):
                        nc.tensor.matmul(psg[m][:, :c1 - c0],
                                         wiht_sb[:, k, m * KP:(m + 1) * KP],
                                         h2t_sb[:, k, c0:c1],
                                         start=(k == 0), stop=(k == 1))
                    nc.scalar.activation(gi_sb[:, m, c0:c1], psg[m][:, :c1 - c0],
                                         AF.Identity, bias=gib_sb[:, m:m + 1])
            # per-core GI patch on the first HALO columns (core 0 kills its pads)
            for m in range(6):
                nc.vector.tensor_scalar(gi_sb[:, m, :HALO], gi_sb[:, m, :HALO],
                                        patch_sb[:, m:m + 1],
                                        patch_sb[:, 6 + m:7 + m],
                                        ALU.mult, ALU.add)

            # ---- GRU fixed-point sweeps ----
            hsh_sb = big.tile([KP, 2, L + 1], f16)
            for mm in range(2):
                nc.vector.memset(hsh_sb[:, mm, :], 0.0)
            # Per sweep: gates from previous-sweep h (Jacobi), then the exact
            # affine scan.  The next sweep's 4 GI-identity matmuls are emitted
            # before the scans so the PE has work during the scan gap (keeps
            # the HAM clock warm).
            prefetched = None
            for s in range(SWEEPS):
                z_sb = big.tile([KP, 2, L], f16, tag="Z")
                b_sb = big.tile([KP, 2, L], f16, tag="B")
                for ci, (c0, c1) in enumerate(ch512):
                    cw = c1 - c0
                    if ci == 0 and prefetched is not None:
                        psg = prefetched
                        prefetched = None
                    else:
                        psg = [pspool.tile([KP, 512], f32, tag=f"g{m}",
                                           name=f"psu_{m}") for m in range(6)]
                        # u_rz = GI_rz (identity matmul) + W_hh_rz @ h_prev
                        for m in range(4):
                            nc.tensor.matmul(psg[m][:, :cw], ident_sb[:],
                                             gi_sb[:, m, c0:c1],
                                             start=True, stop=False)
                    for m in range(6):
                        for k in range(2):
                            nc.tensor.matmul(psg[m][:, :cw],
                                             whht_sb[:, k, m * KP:(m + 1) * KP],
                                             hsh_sb[:, k, c0:c1],
                                             start=(m >= 4 and k == 0),
                                             stop=(k == 1))
                    for mm in range(2):
                        r_t = tpool.tile([KP, 512], f16, tag="r")
                        t_t = tpool.tile([KP, 512], f16, tag="t")
                        un_t = tpool.tile([KP, 512], f16, tag="un")
                        n_t = tpool.tile([KP, 512], f16, tag="n")
                        nc.scalar.activation(r_t[:, :cw], psg[mm][:, :cw],
                                             AF.Sigmoid)
                        nc.scalar.activation(z_sb[:, mm, c0:c1],
                                             psg[2 + mm][:, :cw], AF.Sigmoid)
                        # t = (gh_n + b_hh_n) * r  in one DVE op off PSUM
                        nc.vector.scalar_tensor_tensor(
                            t_t[:, :cw], psg[4 + mm][:, :cw],
                            bhn_sb[:, mm:mm + 1], r_t[:, :cw],
                            ALU.add, ALU.mult)
                        nc.vector.tensor_add(un_t[:, :cw], t_t[:, :cw],
                                             gi_sb[:, 4 + mm, c0:c1])
                        nc.scalar.activation(n_t[:, :cw], un_t[:, :cw], AF.Tanh)
                        # b2 = (z-1)*n; the scan uses op1=subtract so
                        # h = z*h_prev - b2 = z*h_prev + (1-z)*n
                        nc.vector.scalar_tensor_tensor(
                            b_sb[:, mm, c0:c1], z_sb[:, mm, c0:c1], 1.0,
                            n_t[:, :cw], ALU.subtract, ALU.mult)
                        if ci == 2:
                            # keep-warm matmul mid-gates (PE would otherwise be
                            # idle > one HAM window during the gate tail)
                            psd0 = psxw.tile([KP, 512], f32, tag="xwps",
                                             name=f"dwg_{s}_{mm}")
                            nc.tensor.matmul(psd0[:, :cw], ident_sb[:],
                                             z_sb[:, mm, c0:c1],
                                             start=True, stop=True)
                if s < SWEEPS - 1:
                    # PE filler during the scans: next sweep's chunk-0 GI load
                    c0, c1 = ch512[0]
                    psg = [pspool.tile([KP, 512], f32, tag=f"g{m}",
                                       name=f"psp_{m}") for m in range(6)]
                    for m in range(4):
                        nc.tensor.matmul(psg[m][:, :c1 - c0], ident_sb[:],
                                         gi_sb[:, m, c0:c1],
                                         start=True, stop=False)
                    prefetched = psg
                # keep-warm: a throwaway matmul so the PE HAM window does not
                # see a full idle window during the scans
                psd = psxw.tile([KP, 512], f32, tag="xwps", name=f"dwa_{s}")
                nc.tensor.matmul(psd[:, :512], ident_sb[:],
                                 gi_sb[:, 0, 0:512], start=True, stop=True)
                # exact h recurrence: h_t = z_t * h_{t-1} + (1-z_t) n_t
                nc.vector.tensor_tensor_scan(
                    hsh_sb[:, 0, 1:L + 1], z_sb[:, 0, :], b_sb[:, 0, :],
                    0.0, ALU.mult, ALU.subtract)
                psd2 = psxw.tile([KP, 512], f32, tag="xwps", name=f"dwb_{s}")
                nc.tensor.matmul(psd2[:, :512], ident_sb[:],
                                 hsh_sb[:, 0, 0:512], start=True, stop=True)
                nc.vector.tensor_tensor_scan(
                    hsh_sb[:, 1, 1:L + 1], z_sb[:, 1, :], b_sb[:, 1, :],
                    0.0, ALU.mult, ALU.subtract)

            psg_cm.__exit__(None, None, None)

            # ---- final Linear on the real rows (skip halo) ----
            out_sb = cpool.tile([4, ROWS], f32)
            for c0, c1 in _chunks(ROWS):
                cw = c1 - c0
                psf = psxw.tile([KP, 512], f32, tag="xwps")
                for k in range(2):
                    nc.tensor.matmul(psf[:OUT, :cw], fcwt_sb[:, k, :],
                                     hsh_sb[:, k, HALO + 1 + c0:HALO + 1 + c1],
                                     start=(k == 0), stop=(k == 1))
                nc.scalar.activation(out_sb[:OUT, c0:c1], psf[:OUT, :cw],
                                     AF.Identity, bias=fcb_sb[:OUT, :])
            nc.sync.dma_start(out_d[:], out_sb[:OUT, :])

    nc.compile()
    return nc


def host_prepare(inputs):
    """Build the per-core input maps from the full problem inputs."""
    x = np.asarray(inputs["x"], np.float32)
    ei = np.asarray(inputs["edge_index"])
    W1 = np.asarray(inputs["W1"], np.float32)
    b1 = np.asarray(inputs["b1"], np.float32)
    W2 = np.asarray(inputs["W2"], np.float32)
    b2 = np.asarray(inputs["b2"], np.float32)
    W_ih = np.asarray(inputs["W_ih"], np.float32)
    W_hh = np.asarray(inputs["W_hh"], np.float32)
    b_ih = np.asarray(inputs["b_ih"], np.float32)
    b_hh = np.asarray(inputs["b_hh"], np.float32)
    fc_w = np.asarray(inputs["fc_w"], np.float32)
    fc_b = np.asarray(inputs["fc_b"], np.float32)

    N = NUM_NODES
    NPAD = MT * KP
    src, dst = ei[0].astype(np.int64), ei[1].astype(np.int64)
    deg = np.bincount(dst, minlength=N).astype(np.float64) + 1.0
    dinv = 1.0 / np.sqrt(deg)
    # A_T[s, d] = normalization weight of edge s->d (plus self loops)
    at = np.zeros((N, N), np.float32)
    np.add.at(at, (src, dst), (dinv[src] * dinv[dst]).astype(np.float32))
    idx = np.arange(N)
    at[idx, idx] += (dinv * dinv).astype(np.float32)
    at16 = at.astype(np.float16)
    del at

    NPAD = MT * KP
    NPAD2 = MT2 * KP
    xtf = x.T.astype(np.float16)

    common = {
        "w1": W1.astype(np.float16),
        "w2": W2.astype(np.float16),
        "wiht": W_ih.T.astype(np.float16),
        "whht": W_hh.T.astype(np.float16),
        "fcwt": fc_w.T.astype(np.float16),
        "ident": np.eye(KP, dtype=np.float16),
        "b1c": b1.reshape(2, KP).T.astype(np.float32).copy(),
        "b2c": b2.reshape(2, KP).T.astype(np.float32).copy(),
        "gib": (b_ih + np.concatenate([b_hh[:2 * HID],
                                       np.zeros(HID, np.float32)])
                ).reshape(6, KP).T.astype(np.float32).copy(),
        "bhn": b_hh[2 * HID:].reshape(2, KP).T.astype(np.float32).copy(),
        "fcb": np.concatenate([fc_b, np.zeros(KP - OUT, np.float32)]
                              ).reshape(KP, 1),
    }

    in_maps = []
    NPAD3 = GRP * 4 * KP
    for c in range(CORES):
        r0, r1 = c * ROWS, (c + 1) * ROWS
        a2t = np.zeros((NPAD3, L), np.float16)
        if c == 0:
            a2t[:N, HALO:] = at16[:, r0:r1]
        else:
            a2t[:N, :] = at16[:, r0 - HALO:r1]
        # own rows appended at a fixed offset (processed pre-gather with the
        # local XW2 shard); zeroed in the global block to avoid double count
        a2t[NPAD:NPAD + ROWS, :] = a2t[r0:r1, :]
        a2t[r0:r1, :] = 0.0
        # 4-way K-tile interleave: row g*128+p, col j*L+c <- node g*512+j*128+p
        a2t = np.ascontiguousarray(
            a2t.reshape(GRP, 4, KP, L).transpose(0, 2, 1, 3)
        ).reshape(GRP * KP, 4 * L)
        xt = np.zeros((IN_FEAT, NPAD2), np.float16)
        xt[:, :N] = xtf
        xt[:, NPAD:NPAD + ROWS] = xtf[:, r0:r1]
        patch = np.zeros((KP, 12), np.float32)
        if c == 0:
            # mul=0; add=-60 for r,z gate tiles, 0 for n tiles -> pad cols
            # produce exactly h=0 so row 0 starts from the true h0=0.
            patch[:, 6:10] = -60.0
        else:
            patch[:, 0:6] = 1.0
        in_maps.append({**common, "a2t": a2t, "xt": xt, "patch": patch})
    return in_maps


def assemble_output(results):
    outs = [r["out_t"].T for r in results]          # each [ROWS, OUT]
    full = np.concatenate(outs, axis=0).astype(np.float32)
    return full[None]                               # [1, N, OUT]


def kernel(**inputs) -> np.ndarray:
    from concourse import bass_utils

    if "nc" not in _CACHE:
        _CACHE["nc"] = build_program()
    nc = _CACHE["nc"]
    in_maps = host_prepare(inputs)
    res = bass_utils.run_bass_kernel_spmd(
        nc, in_maps, core_ids=list(range(CORES)))
    return assemble_output(res.results)


if __name__ == "__main__":
    import reference

    inputs = {k: np.asarray(v) for k, v in reference.setup_inputs().items()}
    out = kernel(**inputs)
    print("kernel out", out.shape, out.dtype)
    np.save("/root/problem/kernel_out.npy", out)

